# revision 9
# baseline (speedup 1.0000x reference)
"""Birth-death loss kernel v3 for 8 TRN2 NeuronCores.

Same dma_gather structure as v2 (256B blocks per endpoint - the HW
minimum: elem_size bytes must be a multiple of 256) but rebalanced so
the shared DMA engines (the structural bottleneck: 262144 descriptors
x 22.75ns/16 = ~373us/core) are the only near-saturated resource
(~96% busy):

- block indices (i2*8 + j>>6, int16, 16-wrapped, x8 replicated) are
  computed on HOST (pure index layout), killing v2's ~68us of DVE index
  arithmetic and halving the index upload (4MB vs 8MB); w = j&63 ships
  as int8 and is widened to f32 once on DVE;
- chunks of 4096 endpoints per dma_gather call (single_packet=False -
  without it calls above 1024 indices crash the device) cut the 994ns
  SWDGE fixed cost from 256 calls to 64;
- the select is mask-build (iota==w, DVE) + mask*V (mult: Pool for 2 of
  every 3 chunks - the only tensor op walrus codegen accepts on Pool -
  else DVE) + grouped 64->1 reduce (DVE), so Pool (~84%) and DVE (~84%)
  stay under the DMA wall;
- gathers are issued 2 chunks ahead of the Pool mults and idx uploads
  go through a 6-slot ring with paired (2-chunk) DMAs, keeping the DMA
  engines back-to-back; per-stream epilogues run as soon as a stream's
  reduces finish so only the last one sits in the drain tail.

Dependent same-engine DVE chains are drained via a self-semaphore (the
DVE pipeline has no RAW interlock; an op issued back-to-back can read
stale SBUF). Epilogues use the v2 scheme: good-interval corrections via
static slices (good intervals live at fixed k positions). The final
chunk's mask*V multiply is split DVE/Pool to halve the drain tail, and
the kernel returns 128 per-partition partial sums - the host unshard
step sums them (together with +NGOOD per core) exactly as it already
sums the 8 per-core results.

Endpoint order per stream (g=batch, t=interval tensor):
  k = e*32768 + c*8192 + n   (e: 0 birth / 1 death, c: class, n: interval)
"""

import numpy as np

import concourse.bass as bass
import concourse.bacc as bacc
import concourse.mybir as mybir
from concourse.bass_utils import run_bass_kernel_spmd

B, C, H, W, N = 16, 4, 512, 512, 8192
NCORES = 8
BS = B // NCORES               # 2 batches/core
PRED_SZ = BS * C * H * W       # 2097152
G0 = (1, 1, 2, 1)
G1 = (0, 1, 0, 2)
NGOOD = BS * (sum(G0) + sum(G1))

NSTREAM = 4                    # (g, t) pairs: (0,0),(0,1),(1,0),(1,1)
KS = C * N * 2                 # endpoints per stream = 65536
CHUNK = 4096                   # endpoints per dma_gather call
NCH = KS // CHUNK              # 16 chunks per stream
NTOT = NSTREAM * NCH           # 64 chunks total
GPC = CHUNK // 128             # 32 block-columns per chunk
WPC = CHUNK // 128             # 32 w/sel columns per chunk
IXC = CHUNK // 16              # idx cols per chunk = 256

f32 = mybir.dt.float32
i16 = mybir.dt.int16
Alu = mybir.AluOpType
X = mybir.AxisListType.X

STREAMS = [(g, t) for g in range(BS) for t in range(2)]
CNT = {0: G0, 1: G1}

# mult placement: Pool for 2 of 3 chunks (Pool pays 1/0.42 efficiency,
# DVE also carries mask+reduce; this balances both under the DMA wall)
POOL_MULT = [cg % 3 != 0 for cg in range(NTOT)]
# prefix counts: PM[cg] = # pool-mults among chunks 0..cg-1
PM = [0]
for _f in POOL_MULT:
    PM.append(PM[-1] + (1 if _f else 0))
DM = [cg + 1 - PM[cg + 1] for cg in range(-1, NTOT)]  # DM[cg+1] = # dve-mults <= cg


def _build_nc():
    nc = bacc.Bacc(
        "TRN2", target_bir_lowering=False, debug=False, num_devices=NCORES,
        dynamic_dma_scratch_size=3 * 2**15, detect_race_conditions=False,
    )

    pred = nc.dram_tensor("pred", [PRED_SZ // 64, 64], f32, kind="ExternalInput").ap()
    d_ix = [
        nc.dram_tensor(f"ix{s}", [128, KS // 16], i16, kind="ExternalInput").ap()
        for s in range(NSTREAM)
    ]
    d_w8 = nc.dram_tensor("w8", [128, NSTREAM * (KS // 128)], mybir.dt.int8,
                          kind="ExternalInput").ap()
    d_iota = nc.dram_tensor("iotaf", [128, 64], f32, kind="ExternalInput").ap()
    outd = nc.dram_tensor("out", [128, 1], f32, kind="ExternalOutput").ap()

    sb_ixb = nc.alloc_sbuf_tensor("sb_ixb", [128, 6 * IXC], i16).ap()
    sb_ix = [sb_ixb[:, v * IXC:(v + 1) * IXC] for v in range(6)]
    sb_w8 = nc.alloc_sbuf_tensor("sb_w8", [128, NSTREAM * (KS // 128)],
                                 mybir.dt.int8).ap()
    sb_w = [nc.alloc_sbuf_tensor(f"sb_w{s}", [128, KS // 128], f32).ap()
            for s in range(NSTREAM)]
    sb_iota = nc.alloc_sbuf_tensor("sb_iota", [128, 64], f32).ap()
    sb_V = [nc.alloc_sbuf_tensor(f"sb_V{v}", [128, GPC * 64], f32).ap()
            for v in range(4)]
    sb_M = [nc.alloc_sbuf_tensor(f"sb_M{v}", [128, GPC * 64], f32).ap()
            for v in range(2)]
    sb_VM = [nc.alloc_sbuf_tensor(f"sb_VM{v}", [128, GPC * 64], f32).ap()
             for v in range(2)]
    sb_sel = [nc.alloc_sbuf_tensor(f"sb_sel{s}", [128, KS // 128], f32).ap()
              for s in range(NSTREAM)]
    sb_d = [nc.alloc_sbuf_tensor(f"sb_d{s}", [128, KS // 256], f32).ap()
            for s in range(NSTREAM)]
    sb_part = nc.alloc_sbuf_tensor("sb_part", [128, 32], f32).ap()
    sb_S = nc.alloc_sbuf_tensor("sb_S", [128, 1], f32).ap()

    with (
        nc.Block() as block,
        nc.semaphore("dma_in") as dma_in,   # idx chunk uploads (+16 each)
        nc.semaphore("dma_w") as dma_w,     # w/iota uploads (+16 each)
        nc.semaphore("gat") as gat,         # gather DMA completions (+16)
        nc.semaphore("msk") as msk,         # DVE mask built (+1, cg order)
        nc.semaphore("vmP") as vmP,         # Pool mults done (+1)
        nc.semaphore("vmD") as vmD,         # DVE mults done (+1)
        nc.semaphore("red") as red,         # DVE reduces done (+1, cg order)
        nc.semaphore("v_done") as v_done,
        nc.semaphore("vv") as vv,
        nc.semaphore("gat0") as gat0,
    ):

        @block.sync
        def _(sy):
            def ix_upload_pair(cg):
                # uploads chunks cg and cg+1 (same stream: NCH is even) into
                # ring slots cg%6, cg%6+1 with one DMA
                s, c = divmod(cg, NCH)
                if cg >= 6:
                    # slots reused once the gathers 6/5 chunks back drained
                    sy.wait_ge(gat, 16 * (cg - 4))
                sy.dma_start(
                    out=sb_ixb[:, (cg % 6) * IXC:((cg % 6) + 2) * IXC],
                    in_=d_ix[s][:, c * IXC:(c + 2) * IXC],
                ).then_inc(dma_in, 16)

            # first gathers' indices go out before the w tensors so the DMA
            # pipeline fills immediately
            ix_upload_pair(0)
            ix_upload_pair(2)
            sy.dma_start(out=sb_w8, in_=d_w8).then_inc(dma_w, 16)
            sy.dma_start(out=sb_iota, in_=d_iota).then_inc(dma_w, 16)
            for cg in range(4, NTOT, 2):
                ix_upload_pair(cg)
            sy.wait_ge(v_done, 1)
            sy.dma_start(out=outd, in_=sb_S).then_inc(dma_in, 16)

        @block.gpsimd
        def _(g):
            nidx_reg = g.alloc_register("nidx")
            g.reg_mov(nidx_reg, CHUNK)

            def gather_for(cg):
                s, c = divmod(cg, NCH)
                grp = STREAMS[s][0]
                g.wait_ge(dma_in, 16 * (cg // 2 + 1))
                # V[cg%4] free: mult of chunk cg-4 done (Pool order covers
                # pool-mults; wait the DVE sem otherwise)
                if cg >= 4 and not POOL_MULT[cg - 4]:
                    g.wait_ge(vmD, DM[cg - 3])
                src = pred[grp * (PRED_SZ // 128):(grp + 1) * (PRED_SZ // 128), :]
                if cg == 0:
                    # split the first gather 1024+3072 so its (shorter) SWDGE
                    # gets the DMA engines going ~1us earlier; the first sub
                    # signals a throwaway sem so gat still counts one per chunk
                    g.reg_mov(nidx_reg, 1024)
                    g.dma_gather(
                        out_ap=sb_V[0][:, 0:512].rearrange("p (n e) -> p n e", e=64),
                        in_ap=src,
                        idxs_ap=sb_ix[0][:, 0:64],
                        num_idxs=1024,
                        num_idxs_reg=nidx_reg,
                        elem_size=64,
                        single_packet=False,
                    ).then_inc(gat0, 16)
                    g.reg_mov(nidx_reg, 3072)
                    g.dma_gather(
                        out_ap=sb_V[0][:, 512:2048].rearrange("p (n e) -> p n e", e=64),
                        in_ap=src,
                        idxs_ap=sb_ix[0][:, 64:256],
                        num_idxs=3072,
                        num_idxs_reg=nidx_reg,
                        elem_size=64,
                        single_packet=False,
                    ).then_inc(gat, 16)
                    g.reg_mov(nidx_reg, CHUNK)
                    return
                g.dma_gather(
                    out_ap=sb_V[cg % 4].rearrange("p (n e) -> p n e", e=64),
                    in_ap=src,
                    idxs_ap=sb_ix[cg % 6],
                    num_idxs=CHUNK,
                    num_idxs_reg=nidx_reg,
                    elem_size=64,
                    single_packet=False,
                ).then_inc(gat, 16)

            # gathers run 2 chunks ahead of the mults so each mult's waits
            # never delay the next SWDGE (keeps the DMA engines back-to-back)
            gather_for(0)
            gather_for(1)
            for pc in range(NTOT):
                if pc + 2 < NTOT:
                    gather_for(pc + 2)
                if POOL_MULT[pc]:
                    g.wait_ge(gat, 16 * (pc + 1))
                    g.wait_ge(msk, pc + 1)
                    if pc >= 2:
                        g.wait_ge(red, pc - 1)      # VM[pc%2] free
                    g.tensor_tensor(
                        sb_VM[pc % 2], sb_V[pc % 4], sb_M[pc % 2], Alu.mult
                    ).then_inc(vmP, 1)
            # final chunk: Pool takes the second half of the mask*V multiply
            # (in parallel with DVE's first half) to shorten the drain tail
            pc = NTOT - 1
            HFP = GPC * 32
            g.wait_ge(gat, 16 * (pc + 1))
            g.wait_ge(msk, pc + 1)
            g.wait_ge(red, pc - 1)
            g.tensor_tensor(
                sb_VM[pc % 2][:, HFP:2 * HFP], sb_V[pc % 4][:, HFP:2 * HFP],
                sb_M[pc % 2][:, HFP:2 * HFP], Alu.mult
            ).then_inc(vmP, 1)

        @block.vector
        def _(v):
            vc = [0]

            def S(ins):
                # self-sem drain: the DVE pipeline has no RAW interlock, so a
                # dependent op issued back-to-back can read stale SBUF
                vc[0] += 1
                ins.then_inc(vv, 1)
                v.wait_ge(vv, vc[0])
                return ins

            v.memset(sb_part, 0.0)
            v.wait_ge(dma_w, 32)
            WCOL = KS // 128
            for s in range(NSTREAM):
                cp = v.tensor_copy(sb_w[s], sb_w8[:, s * WCOL:(s + 1) * WCOL])
                if s == NSTREAM - 1:
                    S(cp)
            iota_bc = sb_iota.rearrange("p (o e) -> p o e", o=1).broadcast_to(
                [128, GPC, 64]
            )

            def mask_for(cg2):
                s2, c2 = divmod(cg2, NCH)
                w_sl = sb_w[s2][:, c2 * WPC:(c2 + 1) * WPC]
                w_bc = w_sl.unsqueeze(-1).broadcast_to([128, GPC, 64])
                mv = sb_M[cg2 % 2].rearrange("p (n e) -> p n e", e=64)
                # M[cg2%2] free: mult of chunk cg2-2 done (DVE order covers
                # dve-mults; wait the Pool sem otherwise)
                if cg2 >= 2 and POOL_MULT[cg2 - 2]:
                    v.wait_ge(vmP, PM[cg2 - 1])
                v.tensor_tensor(mv, iota_bc, w_bc, Alu.is_equal).then_inc(msk, 1)

            # per-stream epilogue: d = birth - death ; sum d^2 ; corrections
            half = KS // 256   # 256 sel cols per stream half
            ccol = [NSTREAM]

            def epilogue_for(s):
                t = STREAMS[s][1]
                S(v.tensor_tensor(
                    sb_d[s], sb_sel[s][:, 0:half], sb_sel[s][:, half:2 * half],
                    Alu.subtract,
                ))
                S(v.tensor_tensor(sb_sel[s][:, 0:half], sb_d[s], sb_d[s], Alu.mult))
                S(v.tensor_reduce(
                    sb_part[:, s:s + 1], sb_sel[s][:, 0:half], axis=X, op=Alu.add
                ))
                for c4 in range(C):
                    cnt = CNT[t][c4]
                    if cnt == 0:
                        continue
                    dsl = sb_d[s][0:cnt, 64 * c4:64 * c4 + 1]
                    S(v.scalar_tensor_tensor(
                        sb_part[0:cnt, ccol[0]:ccol[0] + 1], dsl, -2.0, dsl,
                        Alu.mult, Alu.mult,
                    ))
                    ccol[0] += 1

            mask_for(0)
            for cg in range(NTOT - 1):
                s, c = divmod(cg, NCH)
                vmv = sb_VM[cg % 2].rearrange("p (n e) -> p n e", e=64)
                if not POOL_MULT[cg]:
                    v.wait_ge(gat, 16 * (cg + 1))
                    if cg >= 2:
                        v.wait_ge(red, cg - 1)      # VM[cg%2] free (own order)
                    v.tensor_tensor(
                        sb_VM[cg % 2], sb_V[cg % 4], sb_M[cg % 2], Alu.mult
                    ).then_inc(vmD, 1)
                mask_for(cg + 1)        # gap op; also mult-pipeline drain
                if POOL_MULT[cg]:
                    v.wait_ge(vmP, PM[cg + 1])
                rins = v.tensor_reduce(
                    sb_sel[s][:, c * WPC:(c + 1) * WPC], vmv, axis=X, op=Alu.add
                )
                rins.then_inc(red, 1)
                if c == NCH - 1:
                    # stream s fully reduced - run its epilogue now (drained
                    # via the reduce's own red increment: red reaches cg+1
                    # only after its writes land) so only the last stream's
                    # epilogue sits in the drain tail
                    v.wait_ge(red, cg + 1)
                    epilogue_for(s)

            # final chunk: DVE multiplies/reduces the first half while Pool
            # multiplies the second; then the last stream's epilogue
            cg = NTOT - 1
            s, c = divmod(cg, NCH)
            HF = GPC * 32
            v.wait_ge(gat, 16 * (cg + 1))
            v.wait_ge(red, cg - 1)
            S(v.tensor_tensor(sb_VM[cg % 2][:, 0:HF], sb_V[cg % 4][:, 0:HF],
                              sb_M[cg % 2][:, 0:HF], Alu.mult))
            v.tensor_reduce(
                sb_sel[s][:, c * WPC:c * WPC + WPC // 2],
                sb_VM[cg % 2][:, 0:HF].rearrange("p (n e) -> p n e", e=64),
                axis=X, op=Alu.add)
            v.wait_ge(vmP, PM[NTOT] + 1)
            S(v.tensor_reduce(
                sb_sel[s][:, c * WPC + WPC // 2:(c + 1) * WPC],
                sb_VM[cg % 2][:, HF:2 * HF].rearrange("p (n e) -> p n e", e=64),
                axis=X, op=Alu.add))
            epilogue_for(s)

            # 128 per-partition partials go back as-is; the host's unshard
            # step sums them together with the per-core +NGOOD constant
            v.tensor_reduce(sb_S, sb_part, axis=X, op=Alu.add).then_inc(v_done, 1)

    nc.compile()
    return nc


_NC = None


def _get_nc():
    global _NC
    if _NC is None:
        _NC = _build_nc()
    return _NC


def _host_prep(iv, t):
    """iv: (BS, C, N, 2, 2) int32 for interval tensor t.
    Returns per-group (ix16 [128, KS//16] i16 block idx, w128 [128, KS//128]
    f32) in the 16-wrapped/replicated and 128-wrapped layouts."""
    outs = []
    for g in range(BS):
        i = iv[g, :, :, :, 0].astype(np.int32)   # (C, N, 2)
        j = iv[g, :, :, :, 1].astype(np.int32)
        i2 = i + 512 * np.arange(C, dtype=np.int32)[:, None, None]
        # k-order: (e, c, n)
        i2k = np.transpose(i2, (2, 0, 1)).reshape(KS)
        jk = np.transpose(j, (2, 0, 1)).reshape(KS)
        blk = (i2k * 8 + (jk >> 6)).astype(np.int16)          # (KS,)
        wrapped = blk.reshape(KS // 16, 16).T                  # (16, KS//16)
        ix16 = np.ascontiguousarray(np.tile(wrapped, (8, 1)))
        w128 = np.ascontiguousarray(
            (jk & 63).astype(np.int8).reshape(KS // 128, 128).T
        )
        outs.append((ix16, w128))
    return outs


def make_in_maps(prediction, intervals_comp_0, intervals_comp_1):
    iotaf = np.tile(np.arange(64, dtype=np.float32), (128, 1))
    in_maps = []
    for m in range(NCORES):
        sl = slice(m * BS, (m + 1) * BS)
        predc = np.ascontiguousarray(
            np.asarray(prediction[sl], dtype=np.float32)
        ).reshape(PRED_SZ // 64, 64)
        prep = {0: _host_prep(np.asarray(intervals_comp_0[sl]), 0),
                1: _host_prep(np.asarray(intervals_comp_1[sl]), 1)}
        im = {"pred": predc, "iotaf": iotaf}
        wparts = []
        for s, (g, t) in enumerate(STREAMS):
            ix16, w128 = prep[t][g]
            im[f"ix{s}"] = ix16
            wparts.append(w128)
        im["w8"] = np.ascontiguousarray(np.concatenate(wparts, axis=1))
        in_maps.append(im)
    return in_maps


def kernel(prediction, intervals_comp_0, intervals_comp_1, **run_kwargs):
    nc = _get_nc()
    in_maps = make_in_maps(prediction, intervals_comp_0, intervals_comp_1)
    res = run_bass_kernel_spmd(nc, in_maps, list(range(NCORES)), **run_kwargs)
    total = 0.0
    for r in res.results:
        total += float(np.sum(r["out"], dtype=np.float64)) + float(NGOOD)
    kernel.last_result = res
    return np.array(total, dtype=np.float32)


# revision 10
# speedup vs baseline: 1.0004x; 1.0004x over previous
"""Birth-death loss kernel v3 for 8 TRN2 NeuronCores.

Same dma_gather structure as v2 (256B blocks per endpoint - the HW
minimum: elem_size bytes must be a multiple of 256) but rebalanced so
the shared DMA engines (the structural bottleneck: 262144 descriptors
x 22.75ns/16 = ~373us/core) are the only near-saturated resource
(~96% busy):

- block indices (i2*8 + j>>6, int16, 16-wrapped, x8 replicated) are
  computed on HOST (pure index layout), killing v2's ~68us of DVE index
  arithmetic and halving the index upload (4MB vs 8MB); w = j&63 ships
  as int8 and is widened to f32 once on DVE;
- chunks of 4096 endpoints per dma_gather call (single_packet=False -
  without it calls above 1024 indices crash the device) cut the 994ns
  SWDGE fixed cost from 256 calls to 64;
- the select is mask-build (iota==w, DVE) + mask*V (mult: Pool for 2 of
  every 3 chunks - the only tensor op walrus codegen accepts on Pool -
  else DVE) + grouped 64->1 reduce (DVE), so Pool (~84%) and DVE (~84%)
  stay under the DMA wall;
- gathers are issued 2 chunks ahead of the Pool mults and idx uploads
  go through a 6-slot ring with paired (2-chunk) DMAs, keeping the DMA
  engines back-to-back; per-stream epilogues run as soon as a stream's
  reduces finish so only the last one sits in the drain tail.

Dependent same-engine DVE chains are drained via a self-semaphore (the
DVE pipeline has no RAW interlock; an op issued back-to-back can read
stale SBUF). Epilogues use the v2 scheme: good-interval corrections via
static slices (good intervals live at fixed k positions). The final
chunk's mask*V multiply is split DVE/Pool to halve the drain tail, and
the kernel returns 128 per-partition partial sums - the host unshard
step sums them (together with +NGOOD per core) exactly as it already
sums the 8 per-core results.

Endpoint order per stream (g=batch, t=interval tensor):
  k = e*32768 + c*8192 + n   (e: 0 birth / 1 death, c: class, n: interval)
"""

import numpy as np

import concourse.bass as bass
import concourse.bacc as bacc
import concourse.mybir as mybir
from concourse.bass_utils import run_bass_kernel_spmd

B, C, H, W, N = 16, 4, 512, 512, 8192
NCORES = 8
BS = B // NCORES               # 2 batches/core
PRED_SZ = BS * C * H * W       # 2097152
G0 = (1, 1, 2, 1)
G1 = (0, 1, 0, 2)
NGOOD = BS * (sum(G0) + sum(G1))

NSTREAM = 4                    # (g, t) pairs: (0,0),(0,1),(1,0),(1,1)
KS = C * N * 2                 # endpoints per stream = 65536
CHUNK = 4096                   # endpoints per dma_gather call
NCH = KS // CHUNK              # 16 chunks per stream
NTOT = NSTREAM * NCH           # 64 chunks total
GPC = CHUNK // 128             # 32 block-columns per chunk
WPC = CHUNK // 128             # 32 w/sel columns per chunk
IXC = CHUNK // 16              # idx cols per chunk = 256

f32 = mybir.dt.float32
i16 = mybir.dt.int16
Alu = mybir.AluOpType
X = mybir.AxisListType.X

STREAMS = [(g, t) for g in range(BS) for t in range(2)]
CNT = {0: G0, 1: G1}

# mult placement: Pool for 2 of 3 chunks (Pool pays 1/0.42 efficiency,
# DVE also carries mask+reduce; this balances both under the DMA wall)
POOL_MULT = [cg % 3 != 0 for cg in range(NTOT)]
# prefix counts: PM[cg] = # pool-mults among chunks 0..cg-1
PM = [0]
for _f in POOL_MULT:
    PM.append(PM[-1] + (1 if _f else 0))
DM = [cg + 1 - PM[cg + 1] for cg in range(-1, NTOT)]  # DM[cg+1] = # dve-mults <= cg


def _build_nc():
    nc = bacc.Bacc(
        "TRN2", target_bir_lowering=False, debug=False, num_devices=NCORES,
        dynamic_dma_scratch_size=3 * 2**15, detect_race_conditions=False,
    )

    pred = nc.dram_tensor("pred", [PRED_SZ // 64, 64], f32, kind="ExternalInput").ap()
    d_ix = [
        nc.dram_tensor(f"ix{s}", [128, KS // 16], i16, kind="ExternalInput").ap()
        for s in range(NSTREAM)
    ]
    d_w8 = nc.dram_tensor("w8", [128, NSTREAM * (KS // 128)], mybir.dt.int8,
                          kind="ExternalInput").ap()
    d_iota = nc.dram_tensor("iotaf", [128, 64], f32, kind="ExternalInput").ap()
    outd = nc.dram_tensor("out", [128, 1], f32, kind="ExternalOutput").ap()

    sb_ixb = nc.alloc_sbuf_tensor("sb_ixb", [128, 6 * IXC], i16).ap()
    sb_ix = [sb_ixb[:, v * IXC:(v + 1) * IXC] for v in range(6)]
    sb_w8 = nc.alloc_sbuf_tensor("sb_w8", [128, NSTREAM * (KS // 128)],
                                 mybir.dt.int8).ap()
    sb_w = [nc.alloc_sbuf_tensor(f"sb_w{s}", [128, KS // 128], f32).ap()
            for s in range(NSTREAM)]
    sb_iota = nc.alloc_sbuf_tensor("sb_iota", [128, 64], f32).ap()
    sb_V = [nc.alloc_sbuf_tensor(f"sb_V{v}", [128, GPC * 64], f32).ap()
            for v in range(4)]
    sb_M = [nc.alloc_sbuf_tensor(f"sb_M{v}", [128, GPC * 64], f32).ap()
            for v in range(2)]
    sb_VM = [nc.alloc_sbuf_tensor(f"sb_VM{v}", [128, GPC * 64], f32).ap()
             for v in range(2)]
    sb_sel = [nc.alloc_sbuf_tensor(f"sb_sel{s}", [128, KS // 128], f32).ap()
              for s in range(NSTREAM)]
    sb_d = [nc.alloc_sbuf_tensor(f"sb_d{s}", [128, KS // 256], f32).ap()
            for s in range(NSTREAM)]
    sb_part = nc.alloc_sbuf_tensor("sb_part", [128, 32], f32).ap()
    sb_S = nc.alloc_sbuf_tensor("sb_S", [128, 1], f32).ap()

    with (
        nc.Block() as block,
        nc.semaphore("dma_in") as dma_in,   # idx chunk uploads (+16 each)
        nc.semaphore("dma_w") as dma_w,     # w/iota uploads (+16 each)
        nc.semaphore("gat") as gat,         # gather DMA completions (+16)
        nc.semaphore("msk") as msk,         # DVE mask built (+1, cg order)
        nc.semaphore("vmP") as vmP,         # Pool mults done (+1)
        nc.semaphore("vmD") as vmD,         # DVE mults done (+1)
        nc.semaphore("red") as red,         # DVE reduces done (+1, cg order)
        nc.semaphore("v_done") as v_done,
        nc.semaphore("vv") as vv,
        nc.semaphore("gat0") as gat0,
    ):

        @block.sync
        def _(sy):
            def ix_upload(cg):
                # single-chunk upload into ring slot cg%6; the slot's last
                # reader is gather cg-6, whose SWDGE is provably done once
                # gather cg-6's DMA drained
                s, c = divmod(cg, NCH)
                if cg >= 6:
                    sy.wait_ge(gat, 16 * (cg - 5))
                sy.dma_start(
                    out=sb_ixb[:, (cg % 6) * IXC:((cg % 6) + 1) * IXC],
                    in_=d_ix[s][:, c * IXC:(c + 1) * IXC],
                ).then_inc(dma_in, 16)

            # first gathers' indices go out before the w tensors so the DMA
            # pipeline fills immediately
            for cg in range(4):
                ix_upload(cg)
            sy.dma_start(out=sb_w8, in_=d_w8).then_inc(dma_w, 16)
            sy.dma_start(out=sb_iota, in_=d_iota).then_inc(dma_w, 16)
            for cg in range(4, NTOT):
                ix_upload(cg)
            sy.wait_ge(v_done, 1)
            sy.dma_start(out=outd, in_=sb_S).then_inc(dma_in, 16)

        @block.gpsimd
        def _(g):
            nidx_reg = g.alloc_register("nidx")
            g.reg_mov(nidx_reg, CHUNK)

            def gather_for(cg):
                s, c = divmod(cg, NCH)
                grp = STREAMS[s][0]
                g.wait_ge(dma_in, 16 * (cg + 1))
                # V[cg%4] free: mult of chunk cg-4 done (Pool order covers
                # pool-mults; wait the DVE sem otherwise)
                if cg >= 4 and not POOL_MULT[cg - 4]:
                    g.wait_ge(vmD, DM[cg - 3])
                src = pred[grp * (PRED_SZ // 128):(grp + 1) * (PRED_SZ // 128), :]
                if cg == 0:
                    # split the first gather 1024+3072 so its (shorter) SWDGE
                    # gets the DMA engines going ~1us earlier; the first sub
                    # signals a throwaway sem so gat still counts one per chunk
                    g.reg_mov(nidx_reg, 1024)
                    g.dma_gather(
                        out_ap=sb_V[0][:, 0:512].rearrange("p (n e) -> p n e", e=64),
                        in_ap=src,
                        idxs_ap=sb_ix[0][:, 0:64],
                        num_idxs=1024,
                        num_idxs_reg=nidx_reg,
                        elem_size=64,
                        single_packet=False,
                    ).then_inc(gat0, 16)
                    g.reg_mov(nidx_reg, 3072)
                    g.dma_gather(
                        out_ap=sb_V[0][:, 512:2048].rearrange("p (n e) -> p n e", e=64),
                        in_ap=src,
                        idxs_ap=sb_ix[0][:, 64:256],
                        num_idxs=3072,
                        num_idxs_reg=nidx_reg,
                        elem_size=64,
                        single_packet=False,
                    ).then_inc(gat, 16)
                    g.reg_mov(nidx_reg, CHUNK)
                    return
                g.dma_gather(
                    out_ap=sb_V[cg % 4].rearrange("p (n e) -> p n e", e=64),
                    in_ap=src,
                    idxs_ap=sb_ix[cg % 6],
                    num_idxs=CHUNK,
                    num_idxs_reg=nidx_reg,
                    elem_size=64,
                    single_packet=False,
                ).then_inc(gat, 16)

            # gathers run 2 chunks ahead of the mults so each mult's waits
            # never delay the next SWDGE (keeps the DMA engines back-to-back)
            gather_for(0)
            gather_for(1)
            for pc in range(NTOT):
                if pc + 2 < NTOT:
                    gather_for(pc + 2)
                if POOL_MULT[pc]:
                    g.wait_ge(gat, 16 * (pc + 1))
                    g.wait_ge(msk, pc + 1)
                    if pc >= 2:
                        g.wait_ge(red, pc - 1)      # VM[pc%2] free
                    g.tensor_tensor(
                        sb_VM[pc % 2], sb_V[pc % 4], sb_M[pc % 2], Alu.mult
                    ).then_inc(vmP, 1)
            # final chunk: Pool takes the second half of the mask*V multiply
            # (in parallel with DVE's first half) to shorten the drain tail
            pc = NTOT - 1
            HFP = GPC * 32
            g.wait_ge(gat, 16 * (pc + 1))
            g.wait_ge(msk, pc + 1)
            g.wait_ge(red, pc - 1)
            g.tensor_tensor(
                sb_VM[pc % 2][:, HFP:2 * HFP], sb_V[pc % 4][:, HFP:2 * HFP],
                sb_M[pc % 2][:, HFP:2 * HFP], Alu.mult
            ).then_inc(vmP, 1)

        @block.vector
        def _(v):
            vc = [0]

            def S(ins):
                # self-sem drain: the DVE pipeline has no RAW interlock, so a
                # dependent op issued back-to-back can read stale SBUF
                vc[0] += 1
                ins.then_inc(vv, 1)
                v.wait_ge(vv, vc[0])
                return ins

            v.memset(sb_part, 0.0)
            v.wait_ge(dma_w, 32)
            WCOL = KS // 128
            for s in range(NSTREAM):
                cp = v.tensor_copy(sb_w[s], sb_w8[:, s * WCOL:(s + 1) * WCOL])
                if s == NSTREAM - 1:
                    S(cp)
            iota_bc = sb_iota.rearrange("p (o e) -> p o e", o=1).broadcast_to(
                [128, GPC, 64]
            )

            def mask_for(cg2):
                s2, c2 = divmod(cg2, NCH)
                w_sl = sb_w[s2][:, c2 * WPC:(c2 + 1) * WPC]
                w_bc = w_sl.unsqueeze(-1).broadcast_to([128, GPC, 64])
                mv = sb_M[cg2 % 2].rearrange("p (n e) -> p n e", e=64)
                # M[cg2%2] free: mult of chunk cg2-2 done (DVE order covers
                # dve-mults; wait the Pool sem otherwise)
                if cg2 >= 2 and POOL_MULT[cg2 - 2]:
                    v.wait_ge(vmP, PM[cg2 - 1])
                v.tensor_tensor(mv, iota_bc, w_bc, Alu.is_equal).then_inc(msk, 1)

            # per-stream epilogue: d = birth - death ; sum d^2 ; corrections
            half = KS // 256   # 256 sel cols per stream half
            ccol = [NSTREAM]

            def epilogue_for(s):
                t = STREAMS[s][1]
                S(v.tensor_tensor(
                    sb_d[s], sb_sel[s][:, 0:half], sb_sel[s][:, half:2 * half],
                    Alu.subtract,
                ))
                S(v.tensor_tensor(sb_sel[s][:, 0:half], sb_d[s], sb_d[s], Alu.mult))
                S(v.tensor_reduce(
                    sb_part[:, s:s + 1], sb_sel[s][:, 0:half], axis=X, op=Alu.add
                ))
                for c4 in range(C):
                    cnt = CNT[t][c4]
                    if cnt == 0:
                        continue
                    dsl = sb_d[s][0:cnt, 64 * c4:64 * c4 + 1]
                    S(v.scalar_tensor_tensor(
                        sb_part[0:cnt, ccol[0]:ccol[0] + 1], dsl, -2.0, dsl,
                        Alu.mult, Alu.mult,
                    ))
                    ccol[0] += 1

            mask_for(0)
            for cg in range(NTOT - 1):
                s, c = divmod(cg, NCH)
                vmv = sb_VM[cg % 2].rearrange("p (n e) -> p n e", e=64)
                if not POOL_MULT[cg]:
                    v.wait_ge(gat, 16 * (cg + 1))
                    if cg >= 2:
                        v.wait_ge(red, cg - 1)      # VM[cg%2] free (own order)
                    v.tensor_tensor(
                        sb_VM[cg % 2], sb_V[cg % 4], sb_M[cg % 2], Alu.mult
                    ).then_inc(vmD, 1)
                mask_for(cg + 1)        # gap op; also mult-pipeline drain
                if POOL_MULT[cg]:
                    v.wait_ge(vmP, PM[cg + 1])
                rins = v.tensor_reduce(
                    sb_sel[s][:, c * WPC:(c + 1) * WPC], vmv, axis=X, op=Alu.add
                )
                rins.then_inc(red, 1)
                if c == NCH - 1:
                    # stream s fully reduced - run its epilogue now (drained
                    # via the reduce's own red increment: red reaches cg+1
                    # only after its writes land) so only the last stream's
                    # epilogue sits in the drain tail
                    v.wait_ge(red, cg + 1)
                    epilogue_for(s)

            # final chunk: DVE multiplies/reduces the first half while Pool
            # multiplies the second; then the last stream's epilogue
            cg = NTOT - 1
            s, c = divmod(cg, NCH)
            HF = GPC * 32
            v.wait_ge(gat, 16 * (cg + 1))
            v.wait_ge(red, cg - 1)
            S(v.tensor_tensor(sb_VM[cg % 2][:, 0:HF], sb_V[cg % 4][:, 0:HF],
                              sb_M[cg % 2][:, 0:HF], Alu.mult))
            v.tensor_reduce(
                sb_sel[s][:, c * WPC:c * WPC + WPC // 2],
                sb_VM[cg % 2][:, 0:HF].rearrange("p (n e) -> p n e", e=64),
                axis=X, op=Alu.add)
            v.wait_ge(vmP, PM[NTOT] + 1)
            S(v.tensor_reduce(
                sb_sel[s][:, c * WPC + WPC // 2:(c + 1) * WPC],
                sb_VM[cg % 2][:, HF:2 * HF].rearrange("p (n e) -> p n e", e=64),
                axis=X, op=Alu.add))
            epilogue_for(s)

            # 128 per-partition partials go back as-is; the host's unshard
            # step sums them together with the per-core +NGOOD constant
            v.tensor_reduce(sb_S, sb_part, axis=X, op=Alu.add).then_inc(v_done, 1)

    nc.compile()
    return nc


_NC = None


def _get_nc():
    global _NC
    if _NC is None:
        _NC = _build_nc()
    return _NC


def _host_prep(iv, t):
    """iv: (BS, C, N, 2, 2) int32 for interval tensor t.
    Returns per-group (ix16 [128, KS//16] i16 block idx, w128 [128, KS//128]
    f32) in the 16-wrapped/replicated and 128-wrapped layouts."""
    outs = []
    for g in range(BS):
        i = iv[g, :, :, :, 0].astype(np.int32)   # (C, N, 2)
        j = iv[g, :, :, :, 1].astype(np.int32)
        i2 = i + 512 * np.arange(C, dtype=np.int32)[:, None, None]
        # k-order: (e, c, n)
        i2k = np.transpose(i2, (2, 0, 1)).reshape(KS)
        jk = np.transpose(j, (2, 0, 1)).reshape(KS)
        blk = (i2k * 8 + (jk >> 6)).astype(np.int16)          # (KS,)
        wrapped = blk.reshape(KS // 16, 16).T                  # (16, KS//16)
        ix16 = np.ascontiguousarray(np.tile(wrapped, (8, 1)))
        w128 = np.ascontiguousarray(
            (jk & 63).astype(np.int8).reshape(KS // 128, 128).T
        )
        outs.append((ix16, w128))
    return outs


def make_in_maps(prediction, intervals_comp_0, intervals_comp_1):
    iotaf = np.tile(np.arange(64, dtype=np.float32), (128, 1))
    in_maps = []
    for m in range(NCORES):
        sl = slice(m * BS, (m + 1) * BS)
        predc = np.ascontiguousarray(
            np.asarray(prediction[sl], dtype=np.float32)
        ).reshape(PRED_SZ // 64, 64)
        prep = {0: _host_prep(np.asarray(intervals_comp_0[sl]), 0),
                1: _host_prep(np.asarray(intervals_comp_1[sl]), 1)}
        im = {"pred": predc, "iotaf": iotaf}
        wparts = []
        for s, (g, t) in enumerate(STREAMS):
            ix16, w128 = prep[t][g]
            im[f"ix{s}"] = ix16
            wparts.append(w128)
        im["w8"] = np.ascontiguousarray(np.concatenate(wparts, axis=1))
        in_maps.append(im)
    return in_maps


def kernel(prediction, intervals_comp_0, intervals_comp_1, **run_kwargs):
    nc = _get_nc()
    in_maps = make_in_maps(prediction, intervals_comp_0, intervals_comp_1)
    res = run_bass_kernel_spmd(nc, in_maps, list(range(NCORES)), **run_kwargs)
    total = 0.0
    for r in res.results:
        total += float(np.sum(r["out"], dtype=np.float64)) + float(NGOOD)
    kernel.last_result = res
    return np.array(total, dtype=np.float32)
